# revision 6
# baseline (speedup 1.0000x reference)
"""Trainium2 8-core multi-head attention kernel (nn_Attention_670014898316).

B=1, S=4096, E=768, H=12 heads of D=64.

Sharding: sequence-parallel over queries (512 rows/core).
- V is projected per-shard and AllGathered (bf16).
- K^T for head pairs 0-3 is computed fully but redundantly on every core
  (this fills the ~100us collective bootstrap+transfer window with useful PE
  work); K^T for pairs 4-5 rides the same AllGather as V.
- Each core then computes its queries' full attention over all 4096 keys for
  all 12 heads plus the output projection; the host concatenates the per-core
  output rows. No all-reduce anywhere.

All matmuls run in bf16 with fp32 PSUM accumulation, in transposed [E, S]
orientation so no on-chip transposes are needed. Softmax skips
max-subtraction and the denominator rides the ctx matmul as a 65th all-ones
row of V.

exp is SPLIT across two engines: head hh=0 of each pair exps on ScalarE
(ACT spline), head hh=1 on the Vector engine via a custom DVE op
EXP2_SQ5_ANT: out = (((x+b)^2+d))^32 with (b,d) fitted so that
q(x)^32 ~ C*2^w for x = gamma*score (gamma baked into ALL heads' Wq
host-side; softmax scale-invariance eats the constant C).  ScalarE exps the
same pre-scaled scores with scale=1/(8*gamma).  This halves the exp wall
(the baseline bottleneck: ~198us serial on ScalarE).

Other perf-critical details learned on silicon:
- PE dual-issues 64-contract score matmuls on opposite row halves; K^T is
  stored with head-halves swapped in alternating 512-column banks and
  GROUP=2 key chunks (one normal + one swapped) guarantee every score
  matmul pair dual-issues.
- Softmax reciprocal: both heads' denominators batched into one [2,512]
  reciprocal_approx (custom DVE, ~5x faster than the iterative divide);
  partition-broadcast + normalize multiplies run on the otherwise-idle
  GpSimd engine.
- ctx PSUM banks are drained to SBUF by ScalarE (it has slack now).
"""

import sys

if "/opt/trn_rl_repo" not in sys.path:
    sys.path.insert(0, "/opt/trn_rl_repo")

import math

import numpy as np
import ml_dtypes

import concourse.bass as bass
import concourse.mybir as mybir
import concourse.tile as tile
from concourse import bacc, bass_utils
from concourse import dve_ops
from concourse.dve_spec import Spec, Src0, C0, C1, sq, lower, _has_src1
from concourse.dve_uop import DveOpSpec

BF16 = mybir.dt.bfloat16
F32 = mybir.dt.float32

B, S, E, H, D = 1, 4096, 768, 12, 64
N_CORES = 8
SC = S // N_CORES          # 512 query rows per core
C = E // 128               # 6 partition chunks of the embedding dim
NPAIR = H // 2             # 6 head pairs
KSZ = E * SC               # elements in one V shard
GROUP = 2                  # score k-chunks per exp instruction

# ---- custom DVE exp op ----------------------------------------------------
# q(x) = (x + b)^2 + d, out = q^32.  With x = score * GAMMA this is
# proportional to exp(score/8) (constant eaten by softmax normalization).
# (b,d) below were fitted (softmax-importance-weighted minimax over the
# actual score range +-14 in log2 units) then normalized so q(0) = 1.
_BHAT, _DHAT = 1.289340, 2.062663
_SBAR = _BHAT * _BHAT + _DHAT
EXP_B = _BHAT / math.sqrt(_SBAR)
EXP_D = _DHAT / _SBAR
LOG2E = 1.4426950408889634
GAMMA = LOG2E / (256.0 * math.sqrt(_SBAR))   # score pre-scale (baked in Wq)
SIG = 1.0 / (8.0 * GAMMA)                    # ScalarE exp scale knob


def _exp2_sq5_ref(in0, in1, s0, s1, imm2):
    q = (in0.astype(np.float32) + np.float32(s0)) ** 2 + np.float32(s1)
    for _ in range(5):
        q = q * q
    return q.astype(np.float32)


def _register_exp_op():
    name = "EXP2_SQ5_ANT"
    for op in dve_ops.OPS:
        if op.name == name:
            return op
    body = sq(sq(sq(sq(sq(sq(Src0 + C0) + C1)))))
    op = dve_ops.DveOp(name, Spec(body=body, reference=_exp2_sq5_ref),
                       subdim=False, uops_sha={})
    dve_ops.OPS.append(op)
    dve_ops.CUSTOM_DVE_SPECS[name] = op.spec
    dve_ops._SUB_OPCODE_FOR_NAME[name] = (
        dve_ops._CUSTOM_DVE_ROW_BASE + len(dve_ops.OPS) - 1)
    for ver in ("v3", "v4"):
        spec_l = DveOpSpec(name=name, opcode=dve_ops.get_dve_sub_opcode(name),
                           uops=lower(op.spec, ver=ver),
                           rd1_en=_has_src1(op.spec))
        op.uops_sha[ver] = spec_l.sha(ver)
    return op


EXP32 = _register_exp_op()


def _build():
    nc = bacc.Bacc("TRN2", target_bir_lowering=False, debug=False,
                   num_devices=N_CORES)

    xqT = nc.dram_tensor("xqT", [E, SC], BF16, kind="ExternalInput")
    xkT = nc.dram_tensor("xkT", [E, S], BF16, kind="ExternalInput")  # FULL keys
    xvT = nc.dram_tensor("xvT", [E, SC], BF16, kind="ExternalInput")
    xksT = nc.dram_tensor("xksT", [E, SC], BF16, kind="ExternalInput")
    wkhT = nc.dram_tensor("wkhT", [E, 256], BF16, kind="ExternalInput")
    wqT = nc.dram_tensor("wqT", [E, E], BF16, kind="ExternalInput")
    wkT = nc.dram_tensor("wkT", [E, E], BF16, kind="ExternalInput")
    wvT = nc.dram_tensor("wvT", [E, E], BF16, kind="ExternalInput")
    woT = nc.dram_tensor("woT", [E, E], BF16, kind="ExternalInput")
    outT = nc.dram_tensor("out", [E, SC], F32, kind="ExternalOutput")

    CCB = KSZ + 256 * SC     # per-rank collective block: V shard + K-third
    cc_in = nc.dram_tensor("cc_in", [CCB], BF16)
    cc_out = nc.dram_tensor("cc_out", [CCB * N_CORES], BF16,
                            addr_space="Shared")
    ccd_in = nc.dram_tensor("ccd_in", [64], BF16)
    ccd_out = nc.dram_tensor("ccd_out", [64 * N_CORES], BF16,
                             addr_space="Shared")

    def load_chunked(pool, dram, ncols, name):
        """Load [E, ncols] dram tensor as [128, C, ncols], one DMA per chunk."""
        t = pool.tile([128, C, ncols], BF16, name=name)
        for ci in range(C):
            nc.sync.dma_start(
                t[:, ci, :],
                bass.AP(tensor=dram, offset=128 * ci * ncols,
                        ap=[[ncols, 128], [1, ncols]]))
        return t

    with tile.TileContext(nc) as tc:
        with (
            tc.tile_pool(name="persist", bufs=1) as persist,
            tc.tile_pool(name="psS", bufs=2, space="PSUM") as psS,
            tc.tile_pool(name="psC", bufs=4, space="PSUM") as psC,
        ):
            # Tiny dummy AllGather first: absorbs the collective entry
            # barrier + ncfw plan staging (~60us) so the real AllGather's
            # trigger delay drops to ~1us.
            with tc.tile_pool(name="dmyp", bufs=1) as dmyp:
                dmy = dmyp.tile([1, 64], BF16, name="dmy")
                nc.vector.memset(dmy[:], 0.0)
                nc.gpsimd.dma_start(ccd_in.ap(), dmy[:])
                nc.gpsimd.collective_compute(
                    "AllGather", mybir.AluOpType.bypass,
                    replica_groups=[list(range(N_CORES))],
                    ins=[ccd_in.ap()], outs=[ccd_out.ap()],
                )

            qT = persist.tile([128, C, SC], BF16)      # Q^T, full per core
            qTs = persist.tile([128, C, SC], BF16)     # partition-swapped Q^T
            ctxT = persist.tile([128, C, SC], BF16)    # normalized context^T
            kT = persist.tile([128, C, S], BF16)       # K^T, FULL (local)
            wo_t = load_chunked(persist, woT, E, "wo_t")

            # ---- Phase A1: V projection + AllGather trigger, Q projection ----
            with tc.tile_pool(name="pa_early", bufs=1) as pa:
                xv_t = load_chunked(pa, xvT, SC, "xv_t")
                wv_t = load_chunked(pa, wvT, E, "wv_t")
                xks_t = load_chunked(pa, xksT, SC, "xks_t")
                wkh_t = pa.tile([128, C, 256], BF16, name="wkh_t")
                for ci in range(C):
                    nc.sync.dma_start(
                        wkh_t[:, ci, :],
                        bass.AP(tensor=wkhT, offset=128 * ci * 256,
                                ap=[[256, 128], [1, 256]]))
                xq_t = load_chunked(pa, xqT, SC, "xq_t")
                wq_t = load_chunked(pa, wqT, E, "wq_t")
                v_sh = pa.tile([128, SC // 128, E], BF16)  # V shard [512, 768]

                for si in range(SC // 128):
                    pt = psS.tile([128, E], F32, tag="S", name="papsum")
                    for n0, n1 in ((0, 512), (512, 768)):
                        for ki in range(C):
                            nc.tensor.matmul(pt[:, n0:n1],
                                             xv_t[:, ki, 128 * si:128 * si + 128],
                                             wv_t[:, ki, n0:n1],
                                             start=(ki == 0), stop=(ki == C - 1))
                    nc.vector.tensor_copy(v_sh[:, si, :], pt[:])
                    nc.sync.dma_start(
                        bass.AP(tensor=cc_in, offset=128 * si * E,
                                ap=[[E, 128], [1, E]]),
                        v_sh[:, si, :])
                for mo2 in range(2):
                    pt = psS.tile([128, E], F32, tag="S", name="papsum")
                    for ki in range(C):
                        nc.tensor.matmul(pt[:, 0:SC],
                                         wkh_t[:, ki, 128 * mo2:128 * mo2 + 128],
                                         xks_t[:, ki, :],
                                         start=(ki == 0), stop=(ki == C - 1))
                    ksh = pa.tile([128, SC], BF16, tag="ksh", name="ksh")
                    nc.vector.tensor_copy(ksh[:], pt[:, 0:SC])
                    nc.sync.dma_start(
                        bass.AP(tensor=cc_in, offset=KSZ + 128 * mo2 * SC,
                                ap=[[SC, 128], [1, SC]]),
                        ksh[:])
                nc.gpsimd.collective_compute(
                    "AllGather", mybir.AluOpType.bypass,
                    replica_groups=[list(range(N_CORES))],
                    ins=[cc_in.ap()], outs=[cc_out.ap()],
                )

                for mo in range(C):
                    pt = psS.tile([128, E], F32, tag="S", name="papsum")
                    for ki in range(C):
                        nc.tensor.matmul(pt[:, 0:SC],
                                         wq_t[:, ki, 128 * mo:128 * mo + 128],
                                         xq_t[:, ki, :],
                                         start=(ki == 0), stop=(ki == C - 1))
                    nc.vector.tensor_copy(qT[:, mo, :], pt[:, 0:SC])
                nc.sync.dma_start(qTs[64:128, :, :], qT[0:64, :, :])
                nc.sync.dma_start(qTs[0:64, :, :], qT[64:128, :, :])

            nchunk = S // 128  # 32 key chunks
            normal = [c for c in range(nchunk) if (c // 4) % 2 == 0]
            swapped = [c for c in range(nchunk) if (c // 4) % 2 == 1]
            order = [c for pair in zip(normal, swapped) for c in pair]
            groups = [order[g:g + GROUP] for g in range(0, nchunk, GROUP)]

            with (
                tc.tile_pool(name="pc_kv", bufs=2) as kv,
                tc.tile_pool(name="pc_pt", bufs=6) as ptp,
                tc.tile_pool(name="pc_misc", bufs=2) as msc,
                tc.tile_pool(name="pc_norm", bufs=1) as nrm,
            ):
                # ---- Phase A2: FULL K^T projection (overlaps AG + phase C) --
                with tc.tile_pool(name="pa_late", bufs=1) as pal:
                    wk_t = load_chunked(pal, wkT, E, "wk_t")
                    wks_t = pal.tile([128, C, E], BF16, name="wks_t")
                    wks_v = wks_t.rearrange("p c (b h e) -> p c b h e", b=C, h=2)
                    wk_v = wk_t.rearrange("p c (b h e) -> p c b h e", b=C, h=2)
                    nc.sync.dma_start(wks_v[:, :, :, 0, :], wk_v[:, :, :, 1, :])
                    nc.sync.dma_start(wks_v[:, :, :, 1, :], wk_v[:, :, :, 0, :])
                    xk_t = pal.tile([128, C, S], BF16, name="xk_t")
                    for ci in range(C):
                        nc.sync.dma_start(
                            xk_t[:, ci, :],
                            bass.AP(tensor=xkT, offset=128 * ci * S,
                                    ap=[[S, 128], [1, S]]))

                    def kfull_block(mo, nb):
                        w_use = wk_t if nb % 2 == 0 else wks_t
                        pt = psS.tile([128, E], F32, tag="S", name="papsum")
                        for ki in range(C):
                            nc.tensor.matmul(
                                pt[:, 0:512],
                                w_use[:, ki, 128 * mo:128 * mo + 128],
                                xk_t[:, ki, 512 * nb:512 * nb + 512],
                                start=(ki == 0), stop=(ki == C - 1))
                        nc.vector.tensor_copy(
                            kT[:, mo, 512 * nb:512 * nb + 512], pt[:, 0:512])

                    for mo in range(4):
                        for nb in range(S // 512):
                            kfull_block(mo, nb)
                    # K rows for pairs 3-5 arrive via the AllGather
                    for mo in range(4, C):
                        nc.sync.dma_start(
                            kT[:, mo, :],
                            bass.AP(tensor=cc_out,
                                    offset=KSZ + (mo - 4) * 128 * SC,
                                    ap=[[SC, 128], [CCB, N_CORES], [1, SC]]))

                    # ---- Phase C: attention, one head-pair at a time ----
                    for h2 in range(NPAIR):
                        # V columns for this pair, ones-augmented: [128, 32, 130]
                        v_p = kv.tile([128, nchunk, 2 * (D + 1)], BF16, tag="v")
                        for r in range(N_CORES):
                            for hh in range(2):
                                nc.sync.dma_start(
                                    v_p[:, 4 * r:4 * r + 4,
                                        (D + 1) * hh:(D + 1) * hh + D],
                                    bass.AP(tensor=cc_out,
                                            offset=(CCB * r + D * (2 * h2 + hh)),
                                            ap=[[E, 128], [128 * E, 4], [1, D]]))
                        ones_view = v_p.rearrange("p c (h e) -> p c h e", h=2)
                        nc.vector.memset(ones_view[:, :, :, D:D + 1], 1.0)

                        ctx = [psC.tile([D + 1, SC], F32, tag="ctx", name=f"ctx{_hh}")
                               for _hh in range(2)]

                        for g in groups:
                            L = len(g)
                            pT0 = ptp.tile([128, GROUP * SC], BF16, tag="pT0",
                                           name="pT0")
                            pT1 = ptp.tile([128, GROUP * SC], BF16, tag="pT1",
                                           name="pT1")
                            Sp = [psS.tile([128, GROUP * SC], F32, tag="S",
                                           name=f"S{_hh}")
                                  for _hh in range(2)]
                            for hh in range(2):
                                for i, kc in enumerate(g):
                                    sw = (kc // 4) % 2
                                    rg = hh ^ sw
                                    p0, p1 = 64 * rg, 64 * rg + 64
                                    q_use = qTs if sw else qT
                                    nc.tensor.matmul(
                                        Sp[hh][:, 512 * i:512 * i + 512],
                                        kT[p0:p1, h2, 128 * kc:128 * kc + 128],
                                        q_use[p0:p1, h2, :],
                                        start=True, stop=True,
                                        tile_position=(64 * rg, 0))
                            # exp: head 0 on ScalarE (spline), head 1 on the
                            # Vector engine (custom DVE poly+5-squarings).
                            nc.scalar.activation(
                                pT0[:, 0:512 * L], Sp[0][:, 0:512 * L],
                                mybir.ActivationFunctionType.Exp, scale=SIG)
                            nc.vector._custom_dve(
                                EXP32, out=pT1[:, 0:512 * L],
                                in0=Sp[1][:, 0:512 * L],
                                s0=EXP_B, s1=EXP_D)
                            for hh, pT in ((0, pT0), (1, pT1)):
                                for i, kc in enumerate(g):
                                    nc.tensor.matmul(
                                        ctx[hh],
                                        v_p[:, kc, (D + 1) * hh:(D + 1) * (hh + 1)],
                                        pT[:, 512 * i:512 * i + 512],
                                        start=(kc == order[0]),
                                        stop=(kc == order[-1]))

                        # drain ctx psum fast (ScalarE has slack), then
                        # normalize from SBUF on GpSimd.
                        cstg = [msc.tile([D + 1, SC], F32, tag="cstg",
                                         name=f"cstg{_hh}") for _hh in range(2)]
                        for hh in range(2):
                            nc.scalar.copy(cstg[hh][:], ctx[hh][:])
                        bc = [nrm.tile([D, SC], F32, tag=f"bc{_hh}",
                                       name=f"bc{_hh}") for _hh in range(2)]
                        for hh in range(2):
                            # custom-DVE ops ignore input partition offsets:
                            # stage the denominator row to partition 0 first.
                            den = nrm.tile([1, SC], F32, tag=f"den{hh}",
                                           name=f"den{hh}")
                            nc.vector.tensor_copy(den[:], cstg[hh][D:D + 1, :])
                            rec = nrm.tile([1, SC], F32, tag=f"rec{hh}",
                                           name=f"rec{hh}")
                            scr = nrm.tile([1, SC], F32, tag=f"scr{hh}",
                                           name=f"scr{hh}")
                            nc.vector.reciprocal_approx_accurate(
                                rec[:], den[:], scr[:])
                            nc.gpsimd.partition_broadcast(
                                bc[hh][:], rec[:], channels=D)
                        nc.gpsimd.tensor_mul(ctxT[0:D, h2, :],
                                             cstg[0][0:D, :], bc[0][:])
                        stg = nrm.tile([D, SC], BF16, tag="stg")
                        nc.gpsimd.tensor_mul(stg[:], cstg[1][0:D, :], bc[1][:])
                        nc.sync.dma_start(ctxT[D:128, h2, :], stg[:])

            # ---------------- Phase D: output projection ----------------
            with tc.tile_pool(name="pd_sb", bufs=2) as pd:
                for mo in range(C):
                    pt = psS.tile([128, SC], F32, tag="S", name="pdpsum")
                    for ki in range(C):
                        nc.tensor.matmul(pt[:, 0:SC],
                                         wo_t[:, ki, 128 * mo:128 * mo + 128],
                                         ctxT[:, ki, :],
                                         start=(ki == 0), stop=(ki == C - 1))
                    st = pd.tile([128, SC], F32, tag="pdst")
                    nc.vector.tensor_copy(st[:], pt[:, 0:SC])
                    nc.sync.dma_start(
                        bass.AP(tensor=outT, offset=128 * mo * SC,
                                ap=[[SC, 128], [1, SC]]),
                        st[:])

    nc.compile()
    return nc


_NC_CACHE = None


def _get_module():
    global _NC_CACHE
    if _NC_CACHE is None:
        _NC_CACHE = _build()
    return _NC_CACHE


def _bf16(a):
    return np.asarray(a, dtype=np.float32).astype(ml_dtypes.bfloat16)


def kernel(inputQueries, inputKeys, inputValues, Wq, Wk, Wv, Wo, _trace=False):
    nc = _get_module()

    # GAMMA is baked into Wq so scores arrive pre-scaled for both exp paths.
    wqT = np.ascontiguousarray(_bf16(np.asarray(Wq, dtype=np.float64).T * GAMMA))
    wkT = np.ascontiguousarray(_bf16(np.asarray(Wk).T))
    swap_idx = np.arange(E).reshape(E // 128, 2, 64)[:, ::-1, :].reshape(E)
    wkTs_host = wkT[:, swap_idx]
    wkh_even = np.ascontiguousarray(wkT[:, 512:768])
    wkh_odd = np.ascontiguousarray(wkTs_host[:, 512:768])
    wvT = np.ascontiguousarray(_bf16(np.asarray(Wv).T))
    woT = np.ascontiguousarray(_bf16(np.asarray(Wo).T))

    xq = np.asarray(inputQueries).reshape(S, E)
    xk = np.asarray(inputKeys).reshape(S, E)
    xv = np.asarray(inputValues).reshape(S, E)
    xkT_full = np.ascontiguousarray(_bf16(xk).T)

    in_maps = []
    for c in range(N_CORES):
        rows = slice(SC * c, SC * (c + 1))
        in_maps.append({
            "xqT": np.ascontiguousarray(_bf16(xq[rows]).T),
            "xkT": xkT_full,
            "xvT": np.ascontiguousarray(_bf16(xv[rows]).T),
            "xksT": np.ascontiguousarray(_bf16(xk[rows]).T),
            "wkhT": wkh_even if c % 2 == 0 else wkh_odd,
            "wqT": wqT, "wkT": wkT, "wvT": wvT, "woT": woT,
        })

    res = bass_utils.run_bass_kernel_spmd(
        nc, in_maps, core_ids=list(range(N_CORES)), trace=_trace)

    out = np.empty((B, S, E), dtype=np.float32)
    for c in range(N_CORES):
        out[0, SC * c:SC * (c + 1), :] = res.results[c]["out"].T
    if _trace:
        return out, res
    return out


# revision 7
# speedup vs baseline: 1.0778x; 1.0778x over previous
"""Trainium2 8-core multi-head attention kernel (nn_Attention_670014898316).

B=1, S=4096, E=768, H=12 heads of D=64.

Sharding: sequence-parallel over queries (512 rows/core).
- V is projected per-shard and AllGathered (bf16).
- K^T for head pairs 0-3 is computed fully but redundantly on every core
  (this fills the ~100us collective bootstrap+transfer window with useful PE
  work); K^T for pairs 4-5 rides the same AllGather as V.
- Each core then computes its queries' full attention over all 4096 keys for
  all 12 heads plus the output projection; the host concatenates the per-core
  output rows. No all-reduce anywhere.

All matmuls run in bf16 with fp32 PSUM accumulation, in transposed [E, S]
orientation so no on-chip transposes are needed. Softmax skips
max-subtraction and the denominator rides the ctx matmul as a 65th all-ones
row of V.

exp is SPLIT across two engines: head hh=0 of each pair exps on ScalarE
(ACT spline), head hh=1 on the Vector engine via a custom DVE op
EXP2_SQ5_ANT: out = (((x+b)^2+d))^32 with (b,d) fitted so that
q(x)^32 ~ C*2^w for x = gamma*score (gamma baked into ALL heads' Wq
host-side; softmax scale-invariance eats the constant C).  ScalarE exps the
same pre-scaled scores with scale=1/(8*gamma).  This halves the exp wall
(the baseline bottleneck: ~198us serial on ScalarE).

Other perf-critical details learned on silicon:
- PE dual-issues 64-contract score matmuls on opposite row halves; K^T is
  stored with head-halves swapped in alternating 512-column banks and
  GROUP=2 key chunks (one normal + one swapped) guarantee every score
  matmul pair dual-issues.
- Softmax reciprocal: both heads' denominators batched into one [2,512]
  reciprocal_approx (custom DVE, ~5x faster than the iterative divide);
  partition-broadcast + normalize multiplies run on the otherwise-idle
  GpSimd engine.
- ctx PSUM banks are drained to SBUF by ScalarE (it has slack now).
"""

import sys

if "/opt/trn_rl_repo" not in sys.path:
    sys.path.insert(0, "/opt/trn_rl_repo")

import math

import numpy as np
import ml_dtypes

import concourse.bass as bass
import concourse.mybir as mybir
import concourse.tile as tile
from concourse import bacc, bass_utils
from concourse import dve_ops
from concourse.dve_spec import Spec, Src0, C0, C1, sq, lower, _has_src1
from concourse.dve_uop import DveOpSpec

BF16 = mybir.dt.bfloat16
F32 = mybir.dt.float32

B, S, E, H, D = 1, 4096, 768, 12, 64
N_CORES = 8
SC = S // N_CORES          # 512 query rows per core
C = E // 128               # 6 partition chunks of the embedding dim
NPAIR = H // 2             # 6 head pairs
KSZ = E * SC               # elements in one V shard
GROUP = 2                  # score k-chunks per exp instruction

# ---- custom DVE exp op ----------------------------------------------------
# q(x) = (x + b)^2 + d, out = q^32.  With x = score * GAMMA this is
# proportional to exp(score/8) (constant eaten by softmax normalization).
# (b,d) below were fitted (softmax-importance-weighted minimax over the
# actual score range +-14 in log2 units) then normalized so q(0) = 1.
_BHAT, _DHAT = 1.289340, 2.062663
_SBAR = _BHAT * _BHAT + _DHAT
EXP_B = _BHAT / math.sqrt(_SBAR)
EXP_D = _DHAT / _SBAR
LOG2E = 1.4426950408889634
GAMMA = LOG2E / (256.0 * math.sqrt(_SBAR))   # score pre-scale (baked in Wq)
SIG = 1.0 / (8.0 * GAMMA)                    # ScalarE exp scale knob


def _exp2_sq5_ref(in0, in1, s0, s1, imm2):
    q = (in0.astype(np.float32) + np.float32(s0)) ** 2 + np.float32(s1)
    for _ in range(5):
        q = q * q
    return q.astype(np.float32)


def _register_exp_op():
    name = "EXP2_SQ5_ANT"
    for op in dve_ops.OPS:
        if op.name == name:
            return op
    body = sq(sq(sq(sq(sq(sq(Src0 + C0) + C1)))))
    op = dve_ops.DveOp(name, Spec(body=body, reference=_exp2_sq5_ref),
                       subdim=False, uops_sha={})
    dve_ops.OPS.append(op)
    dve_ops.CUSTOM_DVE_SPECS[name] = op.spec
    dve_ops._SUB_OPCODE_FOR_NAME[name] = (
        dve_ops._CUSTOM_DVE_ROW_BASE + len(dve_ops.OPS) - 1)
    for ver in ("v3", "v4"):
        spec_l = DveOpSpec(name=name, opcode=dve_ops.get_dve_sub_opcode(name),
                           uops=lower(op.spec, ver=ver),
                           rd1_en=_has_src1(op.spec))
        op.uops_sha[ver] = spec_l.sha(ver)
    return op


EXP32 = _register_exp_op()


def _build():
    nc = bacc.Bacc("TRN2", target_bir_lowering=False, debug=False,
                   num_devices=N_CORES)

    xqT = nc.dram_tensor("xqT", [E, SC], BF16, kind="ExternalInput")
    xkT = nc.dram_tensor("xkT", [E, S], BF16, kind="ExternalInput")  # FULL keys
    xvT = nc.dram_tensor("xvT", [E, SC], BF16, kind="ExternalInput")
    xksT = nc.dram_tensor("xksT", [E, SC], BF16, kind="ExternalInput")
    wkhT = nc.dram_tensor("wkhT", [E, 256], BF16, kind="ExternalInput")
    wqT = nc.dram_tensor("wqT", [E, E], BF16, kind="ExternalInput")
    wkT = nc.dram_tensor("wkT", [E, E], BF16, kind="ExternalInput")
    wvT = nc.dram_tensor("wvT", [E, E], BF16, kind="ExternalInput")
    woT = nc.dram_tensor("woT", [E, E], BF16, kind="ExternalInput")
    outT = nc.dram_tensor("out", [E, SC], F32, kind="ExternalOutput")

    CCB = KSZ + 256 * SC     # per-rank collective block: V shard + K-third
    cc_in = nc.dram_tensor("cc_in", [CCB], BF16)
    cc_out = nc.dram_tensor("cc_out", [CCB * N_CORES], BF16,
                            addr_space="Shared")
    ccd_in = nc.dram_tensor("ccd_in", [64], BF16)
    ccd_out = nc.dram_tensor("ccd_out", [64 * N_CORES], BF16,
                             addr_space="Shared")

    def load_chunked(pool, dram, ncols, name):
        """Load [E, ncols] dram tensor as [128, C, ncols], one DMA per chunk."""
        t = pool.tile([128, C, ncols], BF16, name=name)
        for ci in range(C):
            nc.sync.dma_start(
                t[:, ci, :],
                bass.AP(tensor=dram, offset=128 * ci * ncols,
                        ap=[[ncols, 128], [1, ncols]]))
        return t

    with tile.TileContext(nc) as tc:
        with (
            tc.tile_pool(name="persist", bufs=1) as persist,
            tc.tile_pool(name="rs_dram", bufs=2, space="DRAM") as rs_dram,
            tc.tile_pool(name="psS", bufs=2, space="PSUM") as psS,
            tc.tile_pool(name="psC", bufs=4, space="PSUM") as psC,
        ):
            # Tiny dummy AllGather first: absorbs the collective entry
            # barrier + ncfw plan staging (~60us) so the real AllGather's
            # trigger delay drops to ~1us.
            with tc.tile_pool(name="dmyp", bufs=1) as dmyp:
                dmy = dmyp.tile([1, 64], BF16, name="dmy")
                nc.vector.memset(dmy[:], 0.0)
                nc.gpsimd.dma_start(ccd_in.ap(), dmy[:])
                nc.gpsimd.collective_compute(
                    "AllGather", mybir.AluOpType.bypass,
                    replica_groups=[list(range(N_CORES))],
                    ins=[ccd_in.ap()], outs=[ccd_out.ap()],
                )

            qT = persist.tile([128, C, SC], BF16)      # Q^T, full per core
            qTs = persist.tile([128, C, SC], BF16)     # partition-swapped Q^T
            ctxT = persist.tile([128, C, SC], BF16)    # normalized context^T
            kT = persist.tile([128, C, S], BF16)       # K^T, FULL (local)
            wo_t = load_chunked(persist, woT, E, "wo_t")

            # ---- Phase A1: V projection + AllGather trigger, Q projection ----
            with tc.tile_pool(name="pa_early", bufs=1) as pa:
                xv_t = load_chunked(pa, xvT, SC, "xv_t")
                wv_t = load_chunked(pa, wvT, E, "wv_t")
                xks_t = load_chunked(pa, xksT, SC, "xks_t")
                wkh_t = pa.tile([128, C, 256], BF16, name="wkh_t")
                for ci in range(C):
                    nc.sync.dma_start(
                        wkh_t[:, ci, :],
                        bass.AP(tensor=wkhT, offset=128 * ci * 256,
                                ap=[[256, 128], [1, 256]]))
                xq_t = load_chunked(pa, xqT, SC, "xq_t")
                wq_t = load_chunked(pa, wqT, E, "wq_t")
                v_sh = pa.tile([128, SC // 128, E], BF16)  # V shard [512, 768]

                for si in range(SC // 128):
                    pt = psS.tile([128, E], F32, tag="S", name="papsum")
                    for n0, n1 in ((0, 512), (512, 768)):
                        for ki in range(C):
                            nc.tensor.matmul(pt[:, n0:n1],
                                             xv_t[:, ki, 128 * si:128 * si + 128],
                                             wv_t[:, ki, n0:n1],
                                             start=(ki == 0), stop=(ki == C - 1))
                    nc.vector.tensor_copy(v_sh[:, si, :], pt[:])
                    nc.sync.dma_start(
                        bass.AP(tensor=cc_in, offset=128 * si * E,
                                ap=[[E, 128], [1, E]]),
                        v_sh[:, si, :])
                for mo2 in range(2):
                    pt = psS.tile([128, E], F32, tag="S", name="papsum")
                    for ki in range(C):
                        nc.tensor.matmul(pt[:, 0:SC],
                                         wkh_t[:, ki, 128 * mo2:128 * mo2 + 128],
                                         xks_t[:, ki, :],
                                         start=(ki == 0), stop=(ki == C - 1))
                    ksh = pa.tile([128, SC], BF16, tag="ksh", name="ksh")
                    nc.vector.tensor_copy(ksh[:], pt[:, 0:SC])
                    nc.sync.dma_start(
                        bass.AP(tensor=cc_in, offset=KSZ + 128 * mo2 * SC,
                                ap=[[SC, 128], [1, SC]]),
                        ksh[:])
                nc.gpsimd.collective_compute(
                    "AllGather", mybir.AluOpType.bypass,
                    replica_groups=[list(range(N_CORES))],
                    ins=[cc_in.ap()], outs=[cc_out.ap()],
                )

                for mo in range(C):
                    pt = psS.tile([128, E], F32, tag="S", name="papsum")
                    for ki in range(C):
                        nc.tensor.matmul(pt[:, 0:SC],
                                         wq_t[:, ki, 128 * mo:128 * mo + 128],
                                         xq_t[:, ki, :],
                                         start=(ki == 0), stop=(ki == C - 1))
                    nc.vector.tensor_copy(qT[:, mo, :], pt[:, 0:SC])
                nc.sync.dma_start(qTs[64:128, :, :], qT[0:64, :, :])
                nc.sync.dma_start(qTs[0:64, :, :], qT[64:128, :, :])

            nchunk = S // 128  # 32 key chunks
            normal = [c for c in range(nchunk) if (c // 4) % 2 == 0]
            swapped = [c for c in range(nchunk) if (c // 4) % 2 == 1]
            order = [c for pair in zip(normal, swapped) for c in pair]
            groups = [order[g:g + GROUP] for g in range(0, nchunk, GROUP)]

            with (
                tc.tile_pool(name="pc_kv", bufs=2) as kv,
                tc.tile_pool(name="pc_pt", bufs=6) as ptp,
                tc.tile_pool(name="pc_misc", bufs=2) as msc,
                tc.tile_pool(name="pc_norm", bufs=1) as nrm,
            ):
                # ---- Phase A2: FULL K^T projection (overlaps AG + phase C) --
                with tc.tile_pool(name="pa_late", bufs=1) as pal:
                    wk_t = load_chunked(pal, wkT, E, "wk_t")
                    wks_t = pal.tile([128, C, E], BF16, name="wks_t")
                    wks_v = wks_t.rearrange("p c (b h e) -> p c b h e", b=C, h=2)
                    wk_v = wk_t.rearrange("p c (b h e) -> p c b h e", b=C, h=2)
                    nc.sync.dma_start(wks_v[:, :, :, 0, :], wk_v[:, :, :, 1, :])
                    nc.sync.dma_start(wks_v[:, :, :, 1, :], wk_v[:, :, :, 0, :])
                    xk_t = pal.tile([128, C, S], BF16, name="xk_t")
                    for ci in range(C):
                        nc.sync.dma_start(
                            xk_t[:, ci, :],
                            bass.AP(tensor=xkT, offset=128 * ci * S,
                                    ap=[[S, 128], [1, S]]))

                    def kfull_block(mo, nb):
                        w_use = wk_t if nb % 2 == 0 else wks_t
                        pt = psS.tile([128, E], F32, tag="S", name="papsum")
                        for ki in range(C):
                            nc.tensor.matmul(
                                pt[:, 0:512],
                                w_use[:, ki, 128 * mo:128 * mo + 128],
                                xk_t[:, ki, 512 * nb:512 * nb + 512],
                                start=(ki == 0), stop=(ki == C - 1))
                        nc.vector.tensor_copy(
                            kT[:, mo, 512 * nb:512 * nb + 512], pt[:, 0:512])

                    for mo in range(4):
                        for nb in range(S // 512):
                            kfull_block(mo, nb)
                    # K rows for pairs 3-5 arrive via the AllGather
                    for mo in range(4, C):
                        nc.sync.dma_start(
                            kT[:, mo, :],
                            bass.AP(tensor=cc_out,
                                    offset=KSZ + (mo - 4) * 128 * SC,
                                    ap=[[SC, 128], [CCB, N_CORES], [1, SC]]))

                    # ---- Phase C: attention, one head-pair at a time ----
                    for h2 in range(NPAIR):
                        # V columns for this pair, ones-augmented: [128, 32, 130]
                        v_p = kv.tile([128, nchunk, 2 * (D + 1)], BF16, tag="v")
                        for r in range(N_CORES):
                            for hh in range(2):
                                nc.sync.dma_start(
                                    v_p[:, 4 * r:4 * r + 4,
                                        (D + 1) * hh:(D + 1) * hh + D],
                                    bass.AP(tensor=cc_out,
                                            offset=(CCB * r + D * (2 * h2 + hh)),
                                            ap=[[E, 128], [128 * E, 4], [1, D]]))
                        ones_view = v_p.rearrange("p c (h e) -> p c h e", h=2)
                        nc.vector.memset(ones_view[:, :, :, D:D + 1], 1.0)

                        ctx = [psC.tile([D + 1, SC], F32, tag="ctx", name=f"ctx{_hh}")
                               for _hh in range(2)]

                        def emit_ctx(gg, pT0g, pT1g):
                            for hh, pT in ((0, pT0g), (1, pT1g)):
                                for i, kc in enumerate(gg):
                                    nc.tensor.matmul(
                                        ctx[hh],
                                        v_p[:, kc, (D + 1) * hh:(D + 1) * (hh + 1)],
                                        pT[:, 512 * i:512 * i + 512],
                                        start=(kc == order[0]),
                                        stop=(kc == order[-1]))

                        prev = None
                        for g in groups:
                            L = len(g)
                            pT0 = ptp.tile([128, GROUP * SC], BF16, tag="pT0",
                                           name="pT0")
                            pT1 = ptp.tile([128, GROUP * SC], BF16, tag="pT1",
                                           name="pT1")
                            Sp = [psS.tile([128, GROUP * SC], F32, tag="S",
                                           name=f"S{_hh}")
                                  for _hh in range(2)]
                            for hh in range(2):
                                for i, kc in enumerate(g):
                                    sw = (kc // 4) % 2
                                    rg = hh ^ sw
                                    p0, p1 = 64 * rg, 64 * rg + 64
                                    q_use = qTs if sw else qT
                                    nc.tensor.matmul(
                                        Sp[hh][:, 512 * i:512 * i + 512],
                                        kT[p0:p1, h2, 128 * kc:128 * kc + 128],
                                        q_use[p0:p1, h2, :],
                                        start=True, stop=True,
                                        tile_position=(64 * rg, 0))
                            # exp: head 0 on ScalarE (spline), head 1 on the
                            # Vector engine (custom DVE poly+5-squarings).
                            nc.scalar.activation(
                                pT0[:, 0:512 * L], Sp[0][:, 0:512 * L],
                                mybir.ActivationFunctionType.Exp, scale=SIG)
                            nc.vector._custom_dve(
                                EXP32, out=pT1[:, 0:512 * L],
                                in0=Sp[1][:, 0:512 * L],
                                s0=EXP_B, s1=EXP_D)
                            # ctx of the PREVIOUS group: one-group software
                            # pipeline so the exp latency hides behind a full
                            # group of PE score work.
                            if prev is not None:
                                emit_ctx(*prev)
                            prev = (g, pT0, pT1)
                        emit_ctx(*prev)

                        # drain ctx psum fast (ScalarE has slack), then
                        # normalize from SBUF on GpSimd.
                        cstg = [msc.tile([D + 1, SC], F32, tag="cstg",
                                         name=f"cstg{_hh}") for _hh in range(2)]
                        for hh in range(2):
                            nc.scalar.copy(cstg[hh][:], ctx[hh][:])
                        for hh in range(2):
                            # custom-DVE ops ignore input partition offsets:
                            # stage the denominator row to partition 0 first.
                            den = nrm.tile([1, SC], F32, tag=f"den{hh}",
                                           name=f"den{hh}")
                            nc.vector.tensor_copy(den[:], cstg[hh][D:D + 1, :])
                            rec = nrm.tile([1, SC], F32, tag=f"rec{hh}",
                                           name=f"rec{hh}")
                            scr = nrm.tile([1, SC], F32, tag=f"scr{hh}",
                                           name=f"scr{hh}")
                            nc.vector.reciprocal_approx_accurate(
                                rec[:], den[:], scr[:])
                            rs_b = rs_dram.tile([SC], F32)
                            nc.sync.dma_start(rs_b[:], rec[:])
                            bcast = nrm.tile([D, SC], F32, tag=f"bc{hh}",
                                             name=f"bc{hh}")
                            nc.sync.dma_start(
                                bcast[:],
                                bass.AP(tensor=rs_b.tensor, offset=rs_b.offset,
                                        ap=[[0, D], [1, SC]]))
                            if hh == 0:
                                nc.vector.tensor_mul(ctxT[0:D, h2, :],
                                                     cstg[hh][0:D, :], bcast[:])
                            else:
                                stg = nrm.tile([D, SC], BF16, tag="stg")
                                nc.vector.tensor_mul(stg[:], cstg[hh][0:D, :],
                                                     bcast[:])
                                nc.sync.dma_start(ctxT[D:128, h2, :], stg[:])

            # ---------------- Phase D: output projection ----------------
            with tc.tile_pool(name="pd_sb", bufs=2) as pd:
                for mo in range(C):
                    pt = psS.tile([128, SC], F32, tag="S", name="pdpsum")
                    for ki in range(C):
                        nc.tensor.matmul(pt[:, 0:SC],
                                         wo_t[:, ki, 128 * mo:128 * mo + 128],
                                         ctxT[:, ki, :],
                                         start=(ki == 0), stop=(ki == C - 1))
                    st = pd.tile([128, SC], F32, tag="pdst")
                    nc.vector.tensor_copy(st[:], pt[:, 0:SC])
                    nc.sync.dma_start(
                        bass.AP(tensor=outT, offset=128 * mo * SC,
                                ap=[[SC, 128], [1, SC]]),
                        st[:])

    nc.compile()
    return nc


_NC_CACHE = None


def _get_module():
    global _NC_CACHE
    if _NC_CACHE is None:
        _NC_CACHE = _build()
    return _NC_CACHE


def _bf16(a):
    return np.asarray(a, dtype=np.float32).astype(ml_dtypes.bfloat16)


def kernel(inputQueries, inputKeys, inputValues, Wq, Wk, Wv, Wo, _trace=False):
    nc = _get_module()

    # GAMMA is baked into Wq so scores arrive pre-scaled for both exp paths.
    wqT = np.ascontiguousarray(_bf16(np.asarray(Wq, dtype=np.float64).T * GAMMA))
    wkT = np.ascontiguousarray(_bf16(np.asarray(Wk).T))
    swap_idx = np.arange(E).reshape(E // 128, 2, 64)[:, ::-1, :].reshape(E)
    wkTs_host = wkT[:, swap_idx]
    wkh_even = np.ascontiguousarray(wkT[:, 512:768])
    wkh_odd = np.ascontiguousarray(wkTs_host[:, 512:768])
    wvT = np.ascontiguousarray(_bf16(np.asarray(Wv).T))
    woT = np.ascontiguousarray(_bf16(np.asarray(Wo).T))

    xq = np.asarray(inputQueries).reshape(S, E)
    xk = np.asarray(inputKeys).reshape(S, E)
    xv = np.asarray(inputValues).reshape(S, E)
    xkT_full = np.ascontiguousarray(_bf16(xk).T)

    in_maps = []
    for c in range(N_CORES):
        rows = slice(SC * c, SC * (c + 1))
        in_maps.append({
            "xqT": np.ascontiguousarray(_bf16(xq[rows]).T),
            "xkT": xkT_full,
            "xvT": np.ascontiguousarray(_bf16(xv[rows]).T),
            "xksT": np.ascontiguousarray(_bf16(xk[rows]).T),
            "wkhT": wkh_even if c % 2 == 0 else wkh_odd,
            "wqT": wqT, "wkT": wkT, "wvT": wvT, "woT": woT,
        })

    res = bass_utils.run_bass_kernel_spmd(
        nc, in_maps, core_ids=list(range(N_CORES)), trace=_trace)

    out = np.empty((B, S, E), dtype=np.float32)
    for c in range(N_CORES):
        out[0, SC * c:SC * (c + 1), :] = res.results[c]["out"].T
    if _trace:
        return out, res
    return out


# revision 10
# speedup vs baseline: 1.0803x; 1.0023x over previous
"""Trainium2 8-core multi-head attention kernel (nn_Attention_670014898316).

B=1, S=4096, E=768, H=12 heads of D=64.

Sharding: sequence-parallel over queries (512 rows/core).
- V is projected per-shard and AllGathered (bf16).
- K^T for head pairs 0-3 is computed fully but redundantly on every core
  (this fills the ~100us collective bootstrap+transfer window with useful PE
  work); K^T for pairs 4-5 rides the same AllGather as V.
- Each core then computes its queries' full attention over all 4096 keys for
  all 12 heads plus the output projection; the host concatenates the per-core
  output rows. No all-reduce anywhere.

All matmuls run in bf16 with fp32 PSUM accumulation, in transposed [E, S]
orientation so no on-chip transposes are needed. Softmax skips
max-subtraction and the denominator rides the ctx matmul as a 65th all-ones
row of V.

exp is SPLIT across two engines: head hh=0 of each pair exps on ScalarE
(ACT spline), head hh=1 on the Vector engine via a custom DVE op
EXP2_SQ5_ANT: out = (((x+b)^2+d))^32 with (b,d) fitted so that
q(x)^32 ~ C*2^w for x = gamma*score (gamma baked into ALL heads' Wq
host-side; softmax scale-invariance eats the constant C).  ScalarE exps the
same pre-scaled scores with scale=1/(8*gamma).  This halves the exp wall
(the baseline bottleneck: ~198us serial on ScalarE).

Other perf-critical details learned on silicon:
- PE dual-issues 64-contract score matmuls on opposite row halves; K^T is
  stored with head-halves swapped in alternating 512-column banks and
  GROUP=2 key chunks (one normal + one swapped) guarantee every score
  matmul pair dual-issues.
- Softmax reciprocal: both heads' denominators batched into one [2,512]
  reciprocal_approx (custom DVE, ~5x faster than the iterative divide);
  partition-broadcast + normalize multiplies run on the otherwise-idle
  GpSimd engine.
- ctx PSUM banks are drained to SBUF by ScalarE (it has slack now).
"""

import sys

if "/opt/trn_rl_repo" not in sys.path:
    sys.path.insert(0, "/opt/trn_rl_repo")

import math

import numpy as np
import ml_dtypes

import concourse.bass as bass
import concourse.mybir as mybir
import concourse.tile as tile
from concourse import bacc, bass_utils
from concourse import dve_ops
from concourse.dve_spec import Spec, Src0, C0, C1, sq, lower, _has_src1
from concourse.dve_uop import DveOpSpec

BF16 = mybir.dt.bfloat16
F32 = mybir.dt.float32

B, S, E, H, D = 1, 4096, 768, 12, 64
N_CORES = 8
SC = S // N_CORES          # 512 query rows per core
C = E // 128               # 6 partition chunks of the embedding dim
NPAIR = H // 2             # 6 head pairs
KSZ = E * SC               # elements in one V shard
GROUP = 2                  # score k-chunks per exp instruction

# ---- custom DVE exp op ----------------------------------------------------
# q(x) = (x + b)^2 + d, out = q^32.  With x = score * GAMMA this is
# proportional to exp(score/8) (constant eaten by softmax normalization).
# (b,d) below were fitted (softmax-importance-weighted minimax over the
# actual score range +-14 in log2 units) then normalized so q(0) = 1.
_BHAT, _DHAT = 1.289340, 2.062663
_SBAR = _BHAT * _BHAT + _DHAT
EXP_B = _BHAT / math.sqrt(_SBAR)
EXP_D = _DHAT / _SBAR
LOG2E = 1.4426950408889634
GAMMA = LOG2E / (256.0 * math.sqrt(_SBAR))   # score pre-scale (baked in Wq)
SIG = 1.0 / (8.0 * GAMMA)                    # ScalarE exp scale knob


def _exp2_sq5_ref(in0, in1, s0, s1, imm2):
    q = (in0.astype(np.float32) + np.float32(s0)) ** 2 + np.float32(s1)
    for _ in range(5):
        q = q * q
    return q.astype(np.float32)


def _register_exp_op():
    name = "EXP2_SQ5_ANT"
    for op in dve_ops.OPS:
        if op.name == name:
            return op
    body = sq(sq(sq(sq(sq(sq(Src0 + C0) + C1)))))
    op = dve_ops.DveOp(name, Spec(body=body, reference=_exp2_sq5_ref),
                       subdim=False, uops_sha={})
    dve_ops.OPS.append(op)
    dve_ops.CUSTOM_DVE_SPECS[name] = op.spec
    dve_ops._SUB_OPCODE_FOR_NAME[name] = (
        dve_ops._CUSTOM_DVE_ROW_BASE + len(dve_ops.OPS) - 1)
    for ver in ("v3", "v4"):
        spec_l = DveOpSpec(name=name, opcode=dve_ops.get_dve_sub_opcode(name),
                           uops=lower(op.spec, ver=ver),
                           rd1_en=_has_src1(op.spec))
        op.uops_sha[ver] = spec_l.sha(ver)
    return op


EXP32 = _register_exp_op()


def _build():
    nc = bacc.Bacc("TRN2", target_bir_lowering=False, debug=False,
                   num_devices=N_CORES)

    xqT = nc.dram_tensor("xqT", [E, SC], BF16, kind="ExternalInput")
    xkT = nc.dram_tensor("xkT", [E, S], BF16, kind="ExternalInput")  # FULL keys
    xvT = nc.dram_tensor("xvT", [E, SC], BF16, kind="ExternalInput")
    xksT = nc.dram_tensor("xksT", [E, SC], BF16, kind="ExternalInput")
    wkhT = nc.dram_tensor("wkhT", [E, 256], BF16, kind="ExternalInput")
    wqT = nc.dram_tensor("wqT", [E, E], BF16, kind="ExternalInput")
    wkT = nc.dram_tensor("wkT", [E, E], BF16, kind="ExternalInput")
    wvT = nc.dram_tensor("wvT", [E, E], BF16, kind="ExternalInput")
    woT = nc.dram_tensor("woT", [E, E], BF16, kind="ExternalInput")
    outT = nc.dram_tensor("out", [E, SC], F32, kind="ExternalOutput")

    CCB = KSZ + 256 * SC     # per-rank collective block: V shard + K-third
    cc_in = nc.dram_tensor("cc_in", [CCB], BF16)
    cc_out = nc.dram_tensor("cc_out", [CCB * N_CORES], BF16,
                            addr_space="Shared")
    ccd_in = nc.dram_tensor("ccd_in", [64], BF16)
    ccd_out = nc.dram_tensor("ccd_out", [64 * N_CORES], BF16,
                             addr_space="Shared")

    def load_chunked(pool, dram, ncols, name):
        """Load [E, ncols] dram tensor as [128, C, ncols], one DMA per chunk."""
        t = pool.tile([128, C, ncols], BF16, name=name)
        for ci in range(C):
            nc.sync.dma_start(
                t[:, ci, :],
                bass.AP(tensor=dram, offset=128 * ci * ncols,
                        ap=[[ncols, 128], [1, ncols]]))
        return t

    with tile.TileContext(nc) as tc:
        with (
            tc.tile_pool(name="persist", bufs=1) as persist,
            tc.tile_pool(name="rs_dram", bufs=2, space="DRAM") as rs_dram,
            tc.tile_pool(name="psS", bufs=3, space="PSUM") as psS,
            tc.tile_pool(name="psC", bufs=2, space="PSUM") as psC,
        ):
            # Tiny dummy AllGather first: absorbs the collective entry
            # barrier + ncfw plan staging (~60us) so the real AllGather's
            # trigger delay drops to ~1us.
            with tc.tile_pool(name="dmyp", bufs=1) as dmyp:
                dmy = dmyp.tile([1, 64], BF16, name="dmy")
                nc.vector.memset(dmy[:], 0.0)
                nc.gpsimd.dma_start(ccd_in.ap(), dmy[:])
                nc.gpsimd.collective_compute(
                    "AllGather", mybir.AluOpType.bypass,
                    replica_groups=[list(range(N_CORES))],
                    ins=[ccd_in.ap()], outs=[ccd_out.ap()],
                )

            qT = persist.tile([128, C, SC], BF16)      # Q^T, full per core
            qTs = persist.tile([128, C, SC], BF16)     # partition-swapped Q^T
            ctxT = persist.tile([128, C, SC], BF16)    # normalized context^T
            kT = persist.tile([128, C, S], BF16)       # K^T, FULL (local)
            wo_t = load_chunked(persist, woT, E, "wo_t")

            # ---- Phase A: projections. DMA order: V inputs, Kshard
            # inputs, FULL-K inputs (prefetch), Q inputs.  PE order: V proj,
            # Kshard proj, AllGather trigger, FULL-K proj, Q proj.
            pal_cm = tc.tile_pool(name="pa_late", bufs=1)
            pal = pal_cm.__enter__()
            with tc.tile_pool(name="pa_early", bufs=1) as pa:
                xv_t = load_chunked(pa, xvT, SC, "xv_t")
                wv_t = load_chunked(pa, wvT, E, "wv_t")
                xks_t = load_chunked(pa, xksT, SC, "xks_t")
                wkh_t = pa.tile([128, C, 256], BF16, name="wkh_t")
                for ci in range(C):
                    nc.sync.dma_start(
                        wkh_t[:, ci, :],
                        bass.AP(tensor=wkhT, offset=128 * ci * 256,
                                ap=[[256, 128], [1, 256]]))
                wk_t = load_chunked(pal, wkT, E, "wk_t")
                wks_t = pal.tile([128, C, E], BF16, name="wks_t")
                wks_v = wks_t.rearrange("p c (b h e) -> p c b h e", b=C, h=2)
                wk_v = wk_t.rearrange("p c (b h e) -> p c b h e", b=C, h=2)
                nc.sync.dma_start(wks_v[:, :, :, 0, :], wk_v[:, :, :, 1, :])
                nc.sync.dma_start(wks_v[:, :, :, 1, :], wk_v[:, :, :, 0, :])
                xk_t = pal.tile([128, C, S], BF16, name="xk_t")
                for ci in range(C):
                    nc.sync.dma_start(
                        xk_t[:, ci, :],
                        bass.AP(tensor=xkT, offset=128 * ci * S,
                                ap=[[S, 128], [1, S]]))
                xq_t = load_chunked(pa, xqT, SC, "xq_t")
                wq_t = load_chunked(pa, wqT, E, "wq_t")
                v_sh = pa.tile([128, SC // 128, E], BF16)  # V shard [512, 768]

                for si in range(SC // 128):
                    pt = psS.tile([128, E], F32, tag="S", name="papsum")
                    for n0, n1 in ((0, 512), (512, 768)):
                        for ki in range(C):
                            nc.tensor.matmul(pt[:, n0:n1],
                                             xv_t[:, ki, 128 * si:128 * si + 128],
                                             wv_t[:, ki, n0:n1],
                                             start=(ki == 0), stop=(ki == C - 1))
                    nc.vector.tensor_copy(v_sh[:, si, :], pt[:])
                    nc.sync.dma_start(
                        bass.AP(tensor=cc_in, offset=128 * si * E,
                                ap=[[E, 128], [1, E]]),
                        v_sh[:, si, :])
                for mo2 in range(2):
                    pt = psS.tile([128, E], F32, tag="S", name="papsum")
                    for ki in range(C):
                        nc.tensor.matmul(pt[:, 0:SC],
                                         wkh_t[:, ki, 128 * mo2:128 * mo2 + 128],
                                         xks_t[:, ki, :],
                                         start=(ki == 0), stop=(ki == C - 1))
                    ksh = pa.tile([128, SC], BF16, tag="ksh", name="ksh")
                    nc.vector.tensor_copy(ksh[:], pt[:, 0:SC])
                    nc.sync.dma_start(
                        bass.AP(tensor=cc_in, offset=KSZ + 128 * mo2 * SC,
                                ap=[[SC, 128], [1, SC]]),
                        ksh[:])
                nc.gpsimd.collective_compute(
                    "AllGather", mybir.AluOpType.bypass,
                    replica_groups=[list(range(N_CORES))],
                    ins=[cc_in.ap()], outs=[cc_out.ap()],
                )

                def kfull_block(mo, nb):
                    w_use = wk_t if nb % 2 == 0 else wks_t
                    pt = psS.tile([128, E], F32, tag="S", name="papsum")
                    for ki in range(C):
                        nc.tensor.matmul(
                            pt[:, 0:512],
                            w_use[:, ki, 128 * mo:128 * mo + 128],
                            xk_t[:, ki, 512 * nb:512 * nb + 512],
                            start=(ki == 0), stop=(ki == C - 1))
                    nc.vector.tensor_copy(
                        kT[:, mo, 512 * nb:512 * nb + 512], pt[:, 0:512])

                for mo in range(4):
                    for nb in range(S // 512):
                        kfull_block(mo, nb)

                for mo in range(C):
                    pt = psS.tile([128, E], F32, tag="S", name="papsum")
                    for ki in range(C):
                        nc.tensor.matmul(pt[:, 0:SC],
                                         wq_t[:, ki, 128 * mo:128 * mo + 128],
                                         xq_t[:, ki, :],
                                         start=(ki == 0), stop=(ki == C - 1))
                    nc.vector.tensor_copy(qT[:, mo, :], pt[:, 0:SC])
                nc.sync.dma_start(qTs[64:128, :, :], qT[0:64, :, :])
                nc.sync.dma_start(qTs[0:64, :, :], qT[64:128, :, :])

            nchunk = S // 128  # 32 key chunks
            normal = [c for c in range(nchunk) if (c // 4) % 2 == 0]
            swapped = [c for c in range(nchunk) if (c // 4) % 2 == 1]
            order = [c for pair in zip(normal, swapped) for c in pair]
            groups = [order[g:g + GROUP] for g in range(0, nchunk, GROUP)]

            with (
                tc.tile_pool(name="pc_kv", bufs=2) as kv,
                tc.tile_pool(name="pc_pt", bufs=6) as ptp,
                tc.tile_pool(name="pc_misc", bufs=2) as msc,
                tc.tile_pool(name="pc_norm", bufs=1) as nrm,
            ):
                # K rows for pairs 4-5 arrive via the AllGather
                if True:
                    for mo in range(4, C):
                        nc.sync.dma_start(
                            kT[:, mo, :],
                            bass.AP(tensor=cc_out,
                                    offset=KSZ + (mo - 4) * 128 * SC,
                                    ap=[[SC, 128], [CCB, N_CORES], [1, SC]]))

                    # ---- Phase C: attention, one head-pair at a time ----
                    for h2 in range(NPAIR):
                        # V columns for this pair, ones-augmented: [128, 32, 130]
                        v_p = kv.tile([128, nchunk, 2 * (D + 1)], BF16, tag="v")
                        for r in range(N_CORES):
                            for hh in range(2):
                                nc.sync.dma_start(
                                    v_p[:, 4 * r:4 * r + 4,
                                        (D + 1) * hh:(D + 1) * hh + D],
                                    bass.AP(tensor=cc_out,
                                            offset=(CCB * r + D * (2 * h2 + hh)),
                                            ap=[[E, 128], [128 * E, 4], [1, D]]))
                        ones_view = v_p.rearrange("p c (h e) -> p c h e", h=2)
                        nc.vector.memset(ones_view[:, :, :, D:D + 1], 1.0)

                        ctx = [psC.tile([D + 1, SC], F32, tag="ctx", name=f"ctx{_hh}")
                               for _hh in range(2)]

                        def emit_ctx(gg, pT0g, pT1g):
                            for hh, pT in ((0, pT0g), (1, pT1g)):
                                for i, kc in enumerate(gg):
                                    nc.tensor.matmul(
                                        ctx[hh],
                                        v_p[:, kc, (D + 1) * hh:(D + 1) * (hh + 1)],
                                        pT[:, 512 * i:512 * i + 512],
                                        start=(kc == order[0]),
                                        stop=(kc == order[-1]))

                        prev = None
                        for g in groups:
                            L = len(g)
                            pT0 = ptp.tile([128, GROUP * SC], BF16, tag="pT0",
                                           name="pT0")
                            pT1 = ptp.tile([128, GROUP * SC], BF16, tag="pT1",
                                           name="pT1")
                            Sp = [psS.tile([128, GROUP * SC], F32, tag="S",
                                           name=f"S{_hh}")
                                  for _hh in range(2)]
                            for hh in range(2):
                                for i, kc in enumerate(g):
                                    sw = (kc // 4) % 2
                                    rg = hh ^ sw
                                    p0, p1 = 64 * rg, 64 * rg + 64
                                    q_use = qTs if sw else qT
                                    nc.tensor.matmul(
                                        Sp[hh][:, 512 * i:512 * i + 512],
                                        kT[p0:p1, h2, 128 * kc:128 * kc + 128],
                                        q_use[p0:p1, h2, :],
                                        start=True, stop=True,
                                        tile_position=(64 * rg, 0))
                            # exp: head 0 on ScalarE (spline), head 1 on the
                            # Vector engine (custom DVE poly+5-squarings).
                            nc.scalar.activation(
                                pT0[:, 0:512 * L], Sp[0][:, 0:512 * L],
                                mybir.ActivationFunctionType.Exp, scale=SIG)
                            nc.vector._custom_dve(
                                EXP32, out=pT1[:, 0:512 * L],
                                in0=Sp[1][:, 0:512 * L],
                                s0=EXP_B, s1=EXP_D)
                            # ctx of the PREVIOUS group: one-group software
                            # pipeline so the exp latency hides behind a full
                            # group of PE score work.
                            if prev is not None:
                                emit_ctx(*prev)
                            prev = (g, pT0, pT1)
                        emit_ctx(*prev)

                        # drain ctx psum fast (ScalarE has slack), then
                        # normalize from SBUF on GpSimd.
                        cstg = [msc.tile([D + 1, SC], F32, tag="cstg",
                                         name=f"cstg{_hh}") for _hh in range(2)]
                        for hh in range(2):
                            nc.vector.tensor_copy(cstg[hh][:], ctx[hh][:])
                        for hh in range(2):
                            # custom-DVE ops ignore input partition offsets:
                            # stage the denominator row to partition 0 first.
                            den = nrm.tile([1, SC], F32, tag=f"den{hh}",
                                           name=f"den{hh}")
                            nc.vector.tensor_copy(den[:], cstg[hh][D:D + 1, :])
                            rec = nrm.tile([1, SC], F32, tag=f"rec{hh}",
                                           name=f"rec{hh}")
                            scr = nrm.tile([1, SC], F32, tag=f"scr{hh}",
                                           name=f"scr{hh}")
                            nc.vector.reciprocal_approx_accurate(
                                rec[:], den[:], scr[:])
                            rs_b = rs_dram.tile([SC], F32)
                            nc.sync.dma_start(rs_b[:], rec[:])
                            bcast = nrm.tile([D, SC], F32, tag=f"bc{hh}",
                                             name=f"bc{hh}")
                            nc.sync.dma_start(
                                bcast[:],
                                bass.AP(tensor=rs_b.tensor, offset=rs_b.offset,
                                        ap=[[0, D], [1, SC]]))
                            if hh == 0:
                                nc.vector.tensor_mul(ctxT[0:D, h2, :],
                                                     cstg[hh][0:D, :], bcast[:])
                            else:
                                stg = nrm.tile([D, SC], BF16, tag="stg")
                                nc.vector.tensor_mul(stg[:], cstg[hh][0:D, :],
                                                     bcast[:])
                                nc.sync.dma_start(ctxT[D:128, h2, :], stg[:])

            pal_cm.__exit__(None, None, None)

            # ---------------- Phase D: output projection ----------------
            with tc.tile_pool(name="pd_sb", bufs=2) as pd:
                for mo in range(C):
                    pt = psS.tile([128, SC], F32, tag="S", name="pdpsum")
                    for ki in range(C):
                        nc.tensor.matmul(pt[:, 0:SC],
                                         wo_t[:, ki, 128 * mo:128 * mo + 128],
                                         ctxT[:, ki, :],
                                         start=(ki == 0), stop=(ki == C - 1))
                    st = pd.tile([128, SC], F32, tag="pdst")
                    nc.vector.tensor_copy(st[:], pt[:, 0:SC])
                    nc.sync.dma_start(
                        bass.AP(tensor=outT, offset=128 * mo * SC,
                                ap=[[SC, 128], [1, SC]]),
                        st[:])

    nc.compile()
    return nc


_NC_CACHE = None


def _get_module():
    global _NC_CACHE
    if _NC_CACHE is None:
        _NC_CACHE = _build()
    return _NC_CACHE


def _bf16(a):
    return np.asarray(a, dtype=np.float32).astype(ml_dtypes.bfloat16)


def kernel(inputQueries, inputKeys, inputValues, Wq, Wk, Wv, Wo, _trace=False):
    nc = _get_module()

    # GAMMA is baked into Wq so scores arrive pre-scaled for both exp paths.
    wqT = np.ascontiguousarray(_bf16(np.asarray(Wq, dtype=np.float64).T * GAMMA))
    wkT = np.ascontiguousarray(_bf16(np.asarray(Wk).T))
    swap_idx = np.arange(E).reshape(E // 128, 2, 64)[:, ::-1, :].reshape(E)
    wkTs_host = wkT[:, swap_idx]
    wkh_even = np.ascontiguousarray(wkT[:, 512:768])
    wkh_odd = np.ascontiguousarray(wkTs_host[:, 512:768])
    wvT = np.ascontiguousarray(_bf16(np.asarray(Wv).T))
    woT = np.ascontiguousarray(_bf16(np.asarray(Wo).T))

    xq = np.asarray(inputQueries).reshape(S, E)
    xk = np.asarray(inputKeys).reshape(S, E)
    xv = np.asarray(inputValues).reshape(S, E)
    xkT_full = np.ascontiguousarray(_bf16(xk).T)

    in_maps = []
    for c in range(N_CORES):
        rows = slice(SC * c, SC * (c + 1))
        in_maps.append({
            "xqT": np.ascontiguousarray(_bf16(xq[rows]).T),
            "xkT": xkT_full,
            "xvT": np.ascontiguousarray(_bf16(xv[rows]).T),
            "xksT": np.ascontiguousarray(_bf16(xk[rows]).T),
            "wkhT": wkh_even if c % 2 == 0 else wkh_odd,
            "wqT": wqT, "wkT": wkT, "wvT": wvT, "woT": woT,
        })

    res = bass_utils.run_bass_kernel_spmd(
        nc, in_maps, core_ids=list(range(N_CORES)), trace=_trace)

    out = np.empty((B, S, E), dtype=np.float32)
    for c in range(N_CORES):
        out[0, SC * c:SC * (c + 1), :] = res.results[c]["out"].T
    if _trace:
        return out, res
    return out


# revision 11
# speedup vs baseline: 1.1519x; 1.0663x over previous
"""Trainium2 8-core multi-head attention kernel (nn_Attention_670014898316).

B=1, S=4096, E=768, H=12 heads of D=64.

Sharding: sequence-parallel over queries (512 rows/core).
- V is projected per-shard and AllGathered (bf16).
- K^T for head pairs 0-3 is computed fully but redundantly on every core
  (this fills the ~100us collective bootstrap+transfer window with useful PE
  work); K^T for pairs 4-5 rides the same AllGather as V.
- Each core then computes its queries' full attention over all 4096 keys for
  all 12 heads plus the output projection; the host concatenates the per-core
  output rows. No all-reduce anywhere.

All matmuls run in bf16 with fp32 PSUM accumulation, in transposed [E, S]
orientation so no on-chip transposes are needed. Softmax skips
max-subtraction and the denominator rides the ctx matmul as a 65th all-ones
row of V.

exp is SPLIT across two engines: head hh=0 of each pair exps on ScalarE
(ACT spline), head hh=1 on the Vector engine via a custom DVE op
EXP2_SQ5_ANT: out = (((x+b)^2+d))^32 with (b,d) fitted so that
q(x)^32 ~ C*2^w for x = gamma*score (gamma baked into ALL heads' Wq
host-side; softmax scale-invariance eats the constant C).  ScalarE exps the
same pre-scaled scores with scale=1/(8*gamma).  This halves the exp wall
(the baseline bottleneck: ~198us serial on ScalarE).

Other perf-critical details learned on silicon:
- PE dual-issues 64-contract score matmuls on opposite row halves; K^T is
  stored with head-halves swapped in alternating 512-column banks and
  GROUP=2 key chunks (one normal + one swapped) guarantee every score
  matmul pair dual-issues.
- Softmax reciprocal: both heads' denominators batched into one [2,512]
  reciprocal_approx (custom DVE, ~5x faster than the iterative divide);
  partition-broadcast + normalize multiplies run on the otherwise-idle
  GpSimd engine.
- ctx PSUM banks are drained to SBUF by ScalarE (it has slack now).
"""

import sys

if "/opt/trn_rl_repo" not in sys.path:
    sys.path.insert(0, "/opt/trn_rl_repo")

import math

import numpy as np
import ml_dtypes

import concourse.bass as bass
import concourse.mybir as mybir
import concourse.tile as tile
from concourse import bacc, bass_utils
from concourse import dve_ops
from concourse.dve_spec import Spec, Src0, C0, C1, sq, lower, _has_src1
from concourse.dve_uop import DveOpSpec

BF16 = mybir.dt.bfloat16
F32 = mybir.dt.float32

B, S, E, H, D = 1, 4096, 768, 12, 64
N_CORES = 8
SC = S // N_CORES          # 512 query rows per core
C = E // 128               # 6 partition chunks of the embedding dim
NPAIR = H // 2             # 6 head pairs
KSZ = E * SC               # elements in one V shard
GROUP = 2                  # score k-chunks per exp instruction

# ---- custom DVE exp op ----------------------------------------------------
# q(x) = (x + b)^2 + d, out = q^32.  With x = score * GAMMA this is
# proportional to exp(score/8) (constant eaten by softmax normalization).
# (b,d) below were fitted (softmax-importance-weighted minimax over the
# actual score range +-14 in log2 units) then normalized so q(0) = 1.
_BHAT, _DHAT = 1.289340, 2.062663
_SBAR = _BHAT * _BHAT + _DHAT
EXP_B = _BHAT / math.sqrt(_SBAR)
EXP_D = _DHAT / _SBAR
LOG2E = 1.4426950408889634
GAMMA = LOG2E / (256.0 * math.sqrt(_SBAR))   # score pre-scale (baked in Wq)
SIG = 1.0 / (8.0 * GAMMA)                    # ScalarE exp scale knob


def _exp2_sq5_ref(in0, in1, s0, s1, imm2):
    q = (in0.astype(np.float32) + np.float32(s0)) ** 2 + np.float32(s1)
    for _ in range(5):
        q = q * q
    return q.astype(np.float32)


def _register_exp_op():
    name = "EXP2_SQ5_ANT"
    for op in dve_ops.OPS:
        if op.name == name:
            return op
    body = sq(sq(sq(sq(sq(sq(Src0 + C0) + C1)))))
    op = dve_ops.DveOp(name, Spec(body=body, reference=_exp2_sq5_ref),
                       subdim=False, uops_sha={})
    dve_ops.OPS.append(op)
    dve_ops.CUSTOM_DVE_SPECS[name] = op.spec
    dve_ops._SUB_OPCODE_FOR_NAME[name] = (
        dve_ops._CUSTOM_DVE_ROW_BASE + len(dve_ops.OPS) - 1)
    for ver in ("v3", "v4"):
        spec_l = DveOpSpec(name=name, opcode=dve_ops.get_dve_sub_opcode(name),
                           uops=lower(op.spec, ver=ver),
                           rd1_en=_has_src1(op.spec))
        op.uops_sha[ver] = spec_l.sha(ver)
    return op


EXP32 = _register_exp_op()


def _build():
    nc = bacc.Bacc("TRN2", target_bir_lowering=False, debug=False,
                   num_devices=N_CORES)

    xqT = nc.dram_tensor("xqT", [E, SC], BF16, kind="ExternalInput")
    xkT = nc.dram_tensor("xkT", [E, S], BF16, kind="ExternalInput")  # FULL keys
    xvT = nc.dram_tensor("xvT", [E, SC], BF16, kind="ExternalInput")
    xksT = nc.dram_tensor("xksT", [E, SC], BF16, kind="ExternalInput")
    wkhT = nc.dram_tensor("wkhT", [E, 256], BF16, kind="ExternalInput")
    wqT = nc.dram_tensor("wqT", [E, E], BF16, kind="ExternalInput")
    wkT = nc.dram_tensor("wkT", [E, E], BF16, kind="ExternalInput")
    wvT = nc.dram_tensor("wvT", [E, E], BF16, kind="ExternalInput")
    woT = nc.dram_tensor("woT", [E, E], BF16, kind="ExternalInput")
    outT = nc.dram_tensor("out", [E, SC], F32, kind="ExternalOutput")

    CCB = KSZ + 256 * SC     # per-rank collective block: V shard + K-third
    cc_in = nc.dram_tensor("cc_in", [CCB], BF16)
    cc_out = nc.dram_tensor("cc_out", [CCB * N_CORES], BF16,
                            addr_space="Shared")
    ccd_in = nc.dram_tensor("ccd_in", [64], BF16)
    ccd_out = nc.dram_tensor("ccd_out", [64 * N_CORES], BF16,
                             addr_space="Shared")

    def load_chunked(pool, dram, ncols, name):
        """Load [E, ncols] dram tensor as [128, C, ncols], one DMA per chunk."""
        t = pool.tile([128, C, ncols], BF16, name=name)
        for ci in range(C):
            nc.sync.dma_start(
                t[:, ci, :],
                bass.AP(tensor=dram, offset=128 * ci * ncols,
                        ap=[[ncols, 128], [1, ncols]]))
        return t

    with tile.TileContext(nc) as tc:
        with (
            tc.tile_pool(name="persist", bufs=1) as persist,
            tc.tile_pool(name="rs_dram", bufs=2, space="DRAM") as rs_dram,
            tc.tile_pool(name="psS", bufs=3, space="PSUM") as psS,
            tc.tile_pool(name="psC", bufs=2, space="PSUM") as psC,
        ):
            # Tiny dummy AllGather first: absorbs the collective entry
            # barrier + ncfw plan staging (~60us) so the real AllGather's
            # trigger delay drops to ~1us.
            if True:
                # contents of ccd_in/ccd_out are never read; no need to
                # initialize -- trigger the bootstrap ASAP.
                nc.gpsimd.collective_compute(
                    "AllGather", mybir.AluOpType.bypass,
                    replica_groups=[list(range(N_CORES))],
                    ins=[ccd_in.ap()], outs=[ccd_out.ap()],
                )

            qT = persist.tile([128, C, SC], BF16)      # Q^T, full per core
            qTs = persist.tile([128, C, SC], BF16)     # partition-swapped Q^T
            ctxT = persist.tile([128, C, SC], BF16)    # normalized context^T
            kT = persist.tile([128, C, S], BF16)       # K^T, FULL (local)
            wo_t = load_chunked(persist, woT, E, "wo_t")

            # ---- Phase A: projections. DMA order: V inputs, Kshard
            # inputs, FULL-K inputs (prefetch), Q inputs.  PE order: V proj,
            # Kshard proj, AllGather trigger, FULL-K proj, Q proj.
            pal_cm = tc.tile_pool(name="pa_late", bufs=1)
            pal = pal_cm.__enter__()
            with tc.tile_pool(name="pa_early", bufs=1) as pa:
                xv_t = load_chunked(pa, xvT, SC, "xv_t")
                wv_t = load_chunked(pa, wvT, E, "wv_t")
                xks_t = load_chunked(pa, xksT, SC, "xks_t")
                wkh_t = pa.tile([128, C, 256], BF16, name="wkh_t")
                for ci in range(C):
                    nc.sync.dma_start(
                        wkh_t[:, ci, :],
                        bass.AP(tensor=wkhT, offset=128 * ci * 256,
                                ap=[[256, 128], [1, 256]]))
                wk_t = load_chunked(pal, wkT, E, "wk_t")
                wks_t = pal.tile([128, C, E], BF16, name="wks_t")
                wks_v = wks_t.rearrange("p c (b h e) -> p c b h e", b=C, h=2)
                wk_v = wk_t.rearrange("p c (b h e) -> p c b h e", b=C, h=2)
                nc.sync.dma_start(wks_v[:, :, :, 0, :], wk_v[:, :, :, 1, :])
                nc.sync.dma_start(wks_v[:, :, :, 1, :], wk_v[:, :, :, 0, :])
                xk_t = pal.tile([128, C, S], BF16, name="xk_t")
                for ci in range(C):
                    nc.sync.dma_start(
                        xk_t[:, ci, :],
                        bass.AP(tensor=xkT, offset=128 * ci * S,
                                ap=[[S, 128], [1, S]]))
                xq_t = load_chunked(pa, xqT, SC, "xq_t")
                wq_t = load_chunked(pa, wqT, E, "wq_t")
                v_sh = pa.tile([128, SC // 128, E], BF16)  # V shard [512, 768]

                for si in range(SC // 128):
                    pt = psS.tile([128, E], F32, tag="S", name="papsum")
                    for n0, n1 in ((0, 512), (512, 768)):
                        for ki in range(C):
                            nc.tensor.matmul(pt[:, n0:n1],
                                             xv_t[:, ki, 128 * si:128 * si + 128],
                                             wv_t[:, ki, n0:n1],
                                             start=(ki == 0), stop=(ki == C - 1))
                    nc.vector.tensor_copy(v_sh[:, si, :], pt[:])
                    nc.sync.dma_start(
                        bass.AP(tensor=cc_in, offset=128 * si * E,
                                ap=[[E, 128], [1, E]]),
                        v_sh[:, si, :])
                for mo2 in range(2):
                    pt = psS.tile([128, E], F32, tag="S", name="papsum")
                    for ki in range(C):
                        nc.tensor.matmul(pt[:, 0:SC],
                                         wkh_t[:, ki, 128 * mo2:128 * mo2 + 128],
                                         xks_t[:, ki, :],
                                         start=(ki == 0), stop=(ki == C - 1))
                    ksh = pa.tile([128, SC], BF16, tag="ksh", name="ksh")
                    nc.vector.tensor_copy(ksh[:], pt[:, 0:SC])
                    nc.sync.dma_start(
                        bass.AP(tensor=cc_in, offset=KSZ + 128 * mo2 * SC,
                                ap=[[SC, 128], [1, SC]]),
                        ksh[:])
                nc.gpsimd.collective_compute(
                    "AllGather", mybir.AluOpType.bypass,
                    replica_groups=[list(range(N_CORES))],
                    ins=[cc_in.ap()], outs=[cc_out.ap()],
                )

                def kfull_block(mo, nb):
                    w_use = wk_t if nb % 2 == 0 else wks_t
                    pt = psS.tile([128, E], F32, tag="S", name="papsum")
                    for ki in range(C):
                        nc.tensor.matmul(
                            pt[:, 0:512],
                            w_use[:, ki, 128 * mo:128 * mo + 128],
                            xk_t[:, ki, 512 * nb:512 * nb + 512],
                            start=(ki == 0), stop=(ki == C - 1))
                    nc.vector.tensor_copy(
                        kT[:, mo, 512 * nb:512 * nb + 512], pt[:, 0:512])

                for mo in range(4):
                    for nb in range(S // 512):
                        kfull_block(mo, nb)

                for mo in range(C):
                    pt = psS.tile([128, E], F32, tag="S", name="papsum")
                    for ki in range(C):
                        nc.tensor.matmul(pt[:, 0:SC],
                                         wq_t[:, ki, 128 * mo:128 * mo + 128],
                                         xq_t[:, ki, :],
                                         start=(ki == 0), stop=(ki == C - 1))
                    nc.vector.tensor_copy(qT[:, mo, :], pt[:, 0:SC])
                nc.sync.dma_start(qTs[64:128, :, :], qT[0:64, :, :])
                nc.sync.dma_start(qTs[0:64, :, :], qT[64:128, :, :])

            nchunk = S // 128  # 32 key chunks
            normal = [c for c in range(nchunk) if (c // 4) % 2 == 0]
            swapped = [c for c in range(nchunk) if (c // 4) % 2 == 1]
            order = [c for pair in zip(normal, swapped) for c in pair]
            groups = [order[g:g + GROUP] for g in range(0, nchunk, GROUP)]

            with (
                tc.tile_pool(name="pc_kv", bufs=2) as kv,
                tc.tile_pool(name="pc_pt", bufs=6) as ptp,
                tc.tile_pool(name="pc_misc", bufs=2) as msc,
                tc.tile_pool(name="pc_norm", bufs=1) as nrm,
            ):
                # K rows for pairs 4-5 arrive via the AllGather
                if True:
                    for mo in range(4, C):
                        nc.gpsimd.dma_start(
                            kT[:, mo, :],
                            bass.AP(tensor=cc_out,
                                    offset=KSZ + (mo - 4) * 128 * SC,
                                    ap=[[SC, 128], [CCB, N_CORES], [1, SC]]))

                    # ---- Phase C: attention, one head-pair at a time ----
                    gidx = -1
                    for h2 in range(NPAIR):
                        # V columns for this pair, ones-augmented: [128, 32, 130]
                        v_p = kv.tile([128, nchunk, 2 * (D + 1)], BF16, tag="v")
                        for r in range(N_CORES):
                            for hh in range(2):
                                nc.sync.dma_start(
                                    v_p[:, 4 * r:4 * r + 4,
                                        (D + 1) * hh:(D + 1) * hh + D],
                                    bass.AP(tensor=cc_out,
                                            offset=(CCB * r + D * (2 * h2 + hh)),
                                            ap=[[E, 128], [128 * E, 4], [1, D]]))
                        ones_view = v_p.rearrange("p c (h e) -> p c h e", h=2)
                        nc.vector.memset(ones_view[:, :, :, D:D + 1], 1.0)

                        ctx = [psC.tile([D + 1, SC], F32, tag="ctx", name=f"ctx{_hh}")
                               for _hh in range(2)]

                        def emit_ctx(gg, pT0g, pT1g):
                            for i, kc in enumerate(gg):
                                for hh, pT in ((0, pT0g), (1, pT1g)):
                                    nc.tensor.matmul(
                                        ctx[hh],
                                        v_p[:, kc, (D + 1) * hh:(D + 1) * (hh + 1)],
                                        pT[:, 512 * i:512 * i + 512],
                                        start=(kc == order[0]),
                                        stop=(kc == order[-1]))

                        prev = None
                        for g in groups:
                            L = len(g)
                            pT0 = ptp.tile([128, GROUP * SC], BF16, tag="pT0",
                                           name="pT0")
                            pT1 = ptp.tile([128, GROUP * SC], BF16, tag="pT1",
                                           name="pT1")
                            Sp = [psS.tile([128, GROUP * SC], F32, tag="S",
                                           name=f"S{_hh}")
                                  for _hh in range(2)]
                            for hh in range(2):
                                for i, kc in enumerate(g):
                                    sw = (kc // 4) % 2
                                    rg = hh ^ sw
                                    p0, p1 = 64 * rg, 64 * rg + 64
                                    q_use = qTs if sw else qT
                                    nc.tensor.matmul(
                                        Sp[hh][:, 512 * i:512 * i + 512],
                                        kT[p0:p1, h2, 128 * kc:128 * kc + 128],
                                        q_use[p0:p1, h2, :],
                                        start=True, stop=True,
                                        tile_position=(64 * rg, 0))
                            # exp: head 0 on ScalarE (spline), head 1 on the
                            # Vector engine (custom DVE poly+5-squarings);
                            # every 6th group head 1 goes to ScalarE too to
                            # balance the engines.
                            nc.scalar.activation(
                                pT0[:, 0:512 * L], Sp[0][:, 0:512 * L],
                                mybir.ActivationFunctionType.Exp, scale=SIG)
                            gidx += 1
                            if gidx % 6 == 5:
                                nc.scalar.activation(
                                    pT1[:, 0:512 * L], Sp[1][:, 0:512 * L],
                                    mybir.ActivationFunctionType.Exp, scale=SIG)
                            else:
                                nc.vector._custom_dve(
                                    EXP32, out=pT1[:, 0:512 * L],
                                    in0=Sp[1][:, 0:512 * L],
                                    s0=EXP_B, s1=EXP_D)
                            # ctx of the PREVIOUS group: one-group software
                            # pipeline so the exp latency hides behind a full
                            # group of PE score work.
                            if prev is not None:
                                emit_ctx(*prev)
                            prev = (g, pT0, pT1)
                        emit_ctx(*prev)

                        # drain ctx psum fast (ScalarE has slack), then
                        # normalize from SBUF on GpSimd.
                        cstg = [msc.tile([D + 1, SC], F32, tag="cstg",
                                         name=f"cstg{_hh}") for _hh in range(2)]
                        for hh in range(2):
                            nc.vector.tensor_copy(cstg[hh][:], ctx[hh][:])
                        for hh in range(2):
                            # custom-DVE ops ignore input partition offsets:
                            # stage the denominator row to partition 0 first.
                            den = nrm.tile([1, SC], F32, tag=f"den{hh}",
                                           name=f"den{hh}")
                            nc.gpsimd.tensor_copy(den[:], cstg[hh][D:D + 1, :])
                            rec = nrm.tile([1, SC], F32, tag=f"rec{hh}",
                                           name=f"rec{hh}")
                            scr = nrm.tile([1, SC], F32, tag=f"scr{hh}",
                                           name=f"scr{hh}")
                            nc.vector.reciprocal_approx_accurate(
                                rec[:], den[:], scr[:])
                            rs_b = rs_dram.tile([SC], F32)
                            nc.sync.dma_start(rs_b[:], rec[:])
                            bcast = nrm.tile([D, SC], F32, tag=f"bc{hh}",
                                             name=f"bc{hh}")
                            nc.sync.dma_start(
                                bcast[:],
                                bass.AP(tensor=rs_b.tensor, offset=rs_b.offset,
                                        ap=[[0, D], [1, SC]]))
                            if hh == 0:
                                nc.gpsimd.tensor_mul(ctxT[0:D, h2, :],
                                                     cstg[hh][0:D, :], bcast[:])
                            else:
                                stg = nrm.tile([D, SC], BF16, tag="stg")
                                nc.gpsimd.tensor_mul(stg[:], cstg[hh][0:D, :],
                                                     bcast[:])
                                nc.sync.dma_start(ctxT[D:128, h2, :], stg[:])

            pal_cm.__exit__(None, None, None)

            # ---------------- Phase D: output projection ----------------
            with tc.tile_pool(name="pd_sb", bufs=2) as pd:
                for mo in range(C):
                    pt = psS.tile([128, SC], F32, tag="S", name="pdpsum")
                    for ki in range(C):
                        nc.tensor.matmul(pt[:, 0:SC],
                                         wo_t[:, ki, 128 * mo:128 * mo + 128],
                                         ctxT[:, ki, :],
                                         start=(ki == 0), stop=(ki == C - 1))
                    st = pd.tile([128, SC], F32, tag="pdst")
                    nc.vector.tensor_copy(st[:], pt[:, 0:SC])
                    nc.sync.dma_start(
                        bass.AP(tensor=outT, offset=128 * mo * SC,
                                ap=[[SC, 128], [1, SC]]),
                        st[:])

    nc.compile()
    return nc


_NC_CACHE = None


def _get_module():
    global _NC_CACHE
    if _NC_CACHE is None:
        _NC_CACHE = _build()
    return _NC_CACHE


def _bf16(a):
    return np.asarray(a, dtype=np.float32).astype(ml_dtypes.bfloat16)


def kernel(inputQueries, inputKeys, inputValues, Wq, Wk, Wv, Wo, _trace=False):
    nc = _get_module()

    # GAMMA is baked into Wq so scores arrive pre-scaled for both exp paths.
    wqT = np.ascontiguousarray(_bf16(np.asarray(Wq, dtype=np.float64).T * GAMMA))
    wkT = np.ascontiguousarray(_bf16(np.asarray(Wk).T))
    swap_idx = np.arange(E).reshape(E // 128, 2, 64)[:, ::-1, :].reshape(E)
    wkTs_host = wkT[:, swap_idx]
    wkh_even = np.ascontiguousarray(wkT[:, 512:768])
    wkh_odd = np.ascontiguousarray(wkTs_host[:, 512:768])
    wvT = np.ascontiguousarray(_bf16(np.asarray(Wv).T))
    woT = np.ascontiguousarray(_bf16(np.asarray(Wo).T))

    xq = np.asarray(inputQueries).reshape(S, E)
    xk = np.asarray(inputKeys).reshape(S, E)
    xv = np.asarray(inputValues).reshape(S, E)
    xkT_full = np.ascontiguousarray(_bf16(xk).T)

    in_maps = []
    for c in range(N_CORES):
        rows = slice(SC * c, SC * (c + 1))
        in_maps.append({
            "xqT": np.ascontiguousarray(_bf16(xq[rows]).T),
            "xkT": xkT_full,
            "xvT": np.ascontiguousarray(_bf16(xv[rows]).T),
            "xksT": np.ascontiguousarray(_bf16(xk[rows]).T),
            "wkhT": wkh_even if c % 2 == 0 else wkh_odd,
            "wqT": wqT, "wkT": wkT, "wvT": wvT, "woT": woT,
        })

    res = bass_utils.run_bass_kernel_spmd(
        nc, in_maps, core_ids=list(range(N_CORES)), trace=_trace)

    out = np.empty((B, S, E), dtype=np.float32)
    for c in range(N_CORES):
        out[0, SC * c:SC * (c + 1), :] = res.results[c]["out"].T
    if _trace:
        return out, res
    return out
